# revision 5
# baseline (speedup 1.0000x reference)
"""Trainium2 Bass kernel for nn_MultiHeadAttention (B=2, S=2048, D=1024, H=16).

Sharding: 8 cores = data-parallel over batch (2) x tensor-parallel over heads
(4 groups of 4 heads).  Each core:
  - computes Q^T, K^T (transposed [channels, seq] layout) and V (natural
    [seq, channels] layout, augmented with a ones column per head) for its
    4 heads via fp8 DoubleRow matmuls: inputs and weights are split hi/lo
    e4m3 and three of the four cross terms are accumulated (hi*hi, lo*hi,
    hi*lo), which beats bf16 in both speed (0.5 cycles/row, 2 contraction
    chunks per instruction) and accuracy (~0.1% proj error),
  - runs causal flash attention with *transposed* logits [k, q]; the AV
    product is computed in [q, d] orientation (att block as the stationary
    lhsT) so every AV matmul uses all 128 output partitions; the softmax
    denominator falls out of the ones column of V at free offset 64; AV
    emission is software-pipelined 8 chunks behind the QK/exp stream,
  - normalizes per-query on DVE (free-dim broadcast of the reciprocal),
    transposes ctx back to [ch, q] on the PE (identity-transpose matmuls,
    deferred into the next attention block as PE filler work),
  - multiplies by a row-sharded slice of Wo, producing a partial [D, S]
    output stored in bf16.
Host side: shards/pre-transposes inputs (weights pre-scaled x32 so fp8 lo
residuals stay in the normal range; Wo divided by 32 to compensate; the
1/(8*1024) logit scale is folded into the exp activation), sums the 4
partial Wo products per batch entry in f32 and adds the output bias.

HW-validated pitfalls baked into this design: GPSIMD cannot access PSUM
(all PSUM reads go through DVE/ACT); the TENSOR_MASK custom DVE op
mis-executes on HW (causal boundary handled by a tril add on the PE);
the custom-DVE reciprocal needs SBUF partition-0 sources.
"""

import sys

for _p in ("/opt/trn_rl_repo", "/root/.axon_site/_ro/trn_rl_repo"):
    if _p not in sys.path:
        sys.path.insert(0, _p)

import numpy as np
import ml_dtypes

import concourse.bass as bass  # noqa: F401  (registers engines)
import concourse.mybir as mybir
import concourse.tile as tile
from concourse import bacc
from concourse.bass_utils import run_bass_kernel_spmd

BF16 = ml_dtypes.bfloat16
FP8 = ml_dtypes.float8_e4m3
F32 = np.float32
WSCALE = 32.0   # weight pre-scale keeping fp8 lo-residuals in normal range


def _hilo(a):
    """Split f32 array into (hi, lo) fp8 e4m3 with lo = a - f32(hi)."""
    hi = a.astype(FP8)
    lo = (a - hi.astype(F32)).astype(FP8)
    return hi, lo

B, S, D, H, HD = 2, 2048, 1024, 16, 64
NCORES = 8
GROUPS = NCORES // B        # 4 head groups
HPG = H // GROUPS           # 4 heads per core
DCH = HPG * HD              # 256 channels per core
QT = 512                    # query tile width (free dim)
KC = 128                    # key chunk (partition dim)
NQT, NKC = S // QT, S // KC  # 4, 16
SCALE = 1.0 / 8.0           # 1/sqrt(HD)
NEG_BIG = -1.0e9

_NC_CACHE: dict = {}


def _classify(mask: np.ndarray):
    """Classify each (qtile, kchunk) block of the additive attention mask.

    Returns tuple-of-tuples of (kind, off) with kind in
    {skip, full, diag, gen}; off is the first visible column for diag blocks.
    """
    classes = []
    for qt in range(NQT):
        q0 = qt * QT
        row = []
        for kc in range(NKC):
            k0 = kc * KC
            blk = mask[q0:q0 + QT, k0:k0 + KC]
            if np.all(blk <= -1e8):
                row.append(("skip", 0))
            elif not blk.any():
                row.append(("full", 0))
            else:
                off = k0 - q0
                if 0 <= off < QT:
                    qi = np.arange(q0, q0 + QT)[:, None]
                    ki = np.arange(k0, k0 + KC)[None, :]
                    vis = qi >= ki
                    if (not blk[vis].any()) and np.all(blk[~vis] <= -1e8):
                        row.append(("diag", off))
                        continue
                row.append(("gen", 0))
        classes.append(tuple(row))

    has_gen = any(c[0] == "gen" for r in classes for c in r)
    if has_gen:
        # keep things simple/correct for odd masks: every non-skip block
        # takes the general (full-width + mask add) path
        classes = [
            tuple(("gen", 0) if c[0] in ("diag", "full") else c for c in r)
            for r in classes
        ]
    # first visible chunk of each qtile must cover the full tile width so the
    # accumulating matmul's start=True pass initializes every column
    for r in classes:
        first = next((c for c in r if c[0] != "skip"), None)
        assert first is None or first[1] == 0, "unsupported mask pattern"
    return tuple(tuple(r) for r in classes), has_gen


def _build(classes, has_gen: bool, skip_bias: bool = True):
    f32, bf16 = mybir.dt.float32, mybir.dt.bfloat16
    FT = mybir.ActivationFunctionType

    nc = bacc.Bacc("TRN2", target_bir_lowering=False, debug=False)

    fp8 = mybir.dt.float8e4
    # x inputs arrive pre-transposed, split hi/lo fp8 (error-compensated),
    # chunked into seq tiles; weights likewise (pre-scaled x32 on the host
    # so the lo residuals stay in fp8's normal range)
    xq_d = nc.dram_tensor("xq", [NQT, 128, 2, 8, QT], fp8,
                          kind="ExternalInput")
    xk_d = nc.dram_tensor("xk", [NQT, 128, 2, 8, QT], fp8,
                          kind="ExternalInput")
    xv_d = nc.dram_tensor("xv", [NKC, 128, 2, 8, KC], fp8,
                          kind="ExternalInput")
    wq_d = nc.dram_tensor("wq", [128, 2, 8, DCH], fp8, kind="ExternalInput")
    wk_d = nc.dram_tensor("wk", [128, 2, 8, DCH], fp8, kind="ExternalInput")
    wv_d = nc.dram_tensor("wv", [128, 2, 8, DCH], fp8, kind="ExternalInput")
    wo_d = nc.dram_tensor("wo", [128, 2, D], bf16, kind="ExternalInput")
    bq_d = nc.dram_tensor("bq", [1, DCH], bf16, kind="ExternalInput")
    bk_d = nc.dram_tensor("bk", [1, DCH], bf16, kind="ExternalInput")
    bv_d = nc.dram_tensor("bv", [1, DCH], bf16, kind="ExternalInput")
    pad_d = nc.dram_tensor("pad", [128, NKC], f32, kind="ExternalInput")
    tril_d = nc.dram_tensor("tril", [128, 128], bf16, kind="ExternalInput")
    ident_d = nc.dram_tensor("ident", [128, 128], bf16, kind="ExternalInput")
    maskT_d = None
    if has_gen:
        maskT_d = nc.dram_tensor("maskT", [S, S], f32, kind="ExternalInput")
    out_d = nc.dram_tensor("out", [D, S], bf16, kind="ExternalOutput")

    with tile.TileContext(nc) as tc:
        with (
            tc.tile_pool(name="cpool", bufs=1) as cpool,
            tc.tile_pool(name="spool", bufs=5) as spool,
            tc.tile_pool(name="apool", bufs=12) as apool,
            tc.tile_pool(name="psA", bufs=2, space="PSUM") as psA,
            tc.tile_pool(name="psQK", bufs=2, space="PSUM") as psQK,
            tc.tile_pool(name="psAV", bufs=1, space="PSUM") as psAV,
        ):
            # ---------- constants ----------
            # (DMAs issued after the pipeline-critical weight/x loads: every
            # DMA costs >=500ns of descriptor time on the shared engine)
            ones = cpool.tile([1, 512], bf16)
            nc.gpsimd.memset(ones[:], 1.0)
            tril_sb = cpool.tile([128, 128], bf16)
            ident_sb = cpool.tile([128, 128], bf16)
            pad_sb = cpool.tile([128, NKC], f32)
            bq_sb = cpool.tile([1, DCH], bf16)
            bk_sb = cpool.tile([1, DCH], bf16)
            bv_sb = cpool.tile([1, DCH], bf16)

            # ---------- weights / activations (x in 4 seq chunks) ----------
            def make_x(name):
                return [
                    cpool.tile([128, 2, 8, QT], fp8, name=f"{name}{s4}")
                    for s4 in range(NQT)
                ]

            xk_sb, xq_sb = make_x("xk"), make_x("xq")
            xv_sb = [
                cpool.tile([128, 2, 8, KC], fp8, name=f"xv{st}")
                for st in range(NKC)
            ]
            # interleave weight + input DMAs so each pipeline stage can
            # start as soon as its seq-chunk lands
            # split the first seq-chunk loads so the leading projection's
            # matmuls can start as soon as the first contraction half lands
            wk_sb = cpool.tile([128, 2, 8, DCH], fp8)
            nc.sync.dma_start(wk_sb[:], wk_d[:])
            nc.sync.dma_start(xk_sb[0][:, 0, :, :], xk_d[0, :, 0, :, :])
            nc.sync.dma_start(xk_sb[0][:, 1, :, :], xk_d[0, :, 1, :, :])
            wq_sb = cpool.tile([128, 2, 8, DCH], fp8)
            nc.sync.dma_start(wq_sb[:], wq_d[:])
            nc.sync.dma_start(xq_sb[0][:, 0, :, :], xq_d[0, :, 0, :, :])
            nc.sync.dma_start(xq_sb[0][:, 1, :, :], xq_d[0, :, 1, :, :])
            wv_sb = cpool.tile([128, 2, 8, DCH], fp8)
            nc.sync.dma_start(wv_sb[:], wv_d[:])
            nc.gpsimd.dma_start(pad_sb[:], pad_d[:])
            nc.gpsimd.dma_start(tril_sb[:], tril_d[:])
            for st in range(4):
                nc.sync.dma_start(xv_sb[st][:], xv_d[st])
            nc.sync.dma_start(xk_sb[1][:], xk_d[1])
            nc.sync.dma_start(xq_sb[1][:], xq_d[1])
            nc.gpsimd.dma_start(ident_sb[:], ident_d[:])
            if not skip_bias:
                nc.gpsimd.dma_start(bq_sb[:], bq_d[:])
                nc.gpsimd.dma_start(bk_sb[:], bk_d[:])
                nc.gpsimd.dma_start(bv_sb[:], bv_d[:])
            nc.sync.dma_start(xv_sb[4][:], xv_d[4])
            nc.sync.dma_start(xv_sb[5][:], xv_d[5])
            nc.sync.dma_start(xv_sb[6][:], xv_d[6])
            nc.sync.dma_start(xv_sb[7][:], xv_d[7])
            for s4 in range(2, NQT):
                nc.sync.dma_start(xk_sb[s4][:], xk_d[s4])
                nc.sync.dma_start(xq_sb[s4][:], xq_d[s4])
                for st in range(4 * s4, 4 * s4 + 4):
                    nc.sync.dma_start(xv_sb[st][:], xv_d[st])
            wo_sb = cpool.tile([128, 2, D], bf16)
            nc.sync.dma_start(wo_sb[:], wo_d[:])

            qT_sb = cpool.tile([128, 2, S], bf16)   # [pair-stacked ch, pair, s]
            kT_sb = cpool.tile([128, 2, S], bf16)
            v_sb = cpool.tile([128, NKC, HPG, HD + 1], bf16)  # ones col at 64
            nc.gpsimd.memset(v_sb[:], 1.0)  # preset ones columns
            ctxT_sb = cpool.tile([128, 2, S], bf16)

            # ---------- projections (one PSUM group per call) ----------
            DR = mybir.MatmulPerfMode.DoubleRow
            TERMS = ((0, 0), (1, 0), (0, 1))  # (x hi/lo, w hi/lo)

            def proj_qk_g(w_sb, b_sb, x_sb, dst, m, s4):
                ps = psA.tile([128, 512], f32, tag="proj",
                              name=f"pp{id(w_sb) % 97}_{m}{s4}")
                n = 0
                for xt_, wt_ in TERMS:
                    for c2 in range(4):
                        nc.tensor.matmul(
                            ps[:],
                            w_sb[:, wt_, 2 * c2:2 * c2 + 2,
                                 m * 128:(m + 1) * 128],
                            x_sb[s4][:, xt_, 2 * c2:2 * c2 + 2, :],
                            start=(n == 0),
                            stop=(n == 11 and skip_bias),
                            perf_mode=DR,
                        )
                        n += 1
                if not skip_bias:
                    nc.tensor.matmul(
                        ps[:],
                        b_sb[0:1, m * 128:(m + 1) * 128],
                        ones[0:1, 0:512],
                        start=False, stop=True,
                    )
                nc.vector.tensor_copy(
                    dst[:, m, s4 * 512:(s4 + 1) * 512], ps[:]
                )

            def proj_v_g(st):
                ps = psA.tile([128, 512], f32, tag="proj", name=f"pv{st}")
                pv = ps[:, 0:DCH]
                xt = xv_sb[st]
                n = 0
                for xt_, wt_ in TERMS:
                    for c2 in range(4):
                        nc.tensor.matmul(
                            pv,
                            xt[:, xt_, 2 * c2:2 * c2 + 2, :],
                            wv_sb[:, wt_, 2 * c2:2 * c2 + 2, :],
                            start=(n == 0),
                            stop=(n == 11 and skip_bias),
                            perf_mode=DR,
                        )
                        n += 1
                if not skip_bias:
                    nc.tensor.matmul(
                        pv, ones[0:1, 0:128], bv_sb[0:1, :],
                        start=False, stop=True,
                    )
                # NB: ACT mis-executes this strided 3D copy; keep it on DVE
                nc.vector.tensor_copy(
                    v_sb[:, st, :, 0:HD],
                    ps[:, 0:DCH].rearrange("p (h d) -> p h d", h=HPG),
                )

            # ---------- attention for one (qtile, pair) ----------
            from collections import deque

            def attn(qt, p_, filler=None):
                q0 = qt * QT
                chunks = [
                    (kc, classes[qt][kc])
                    for kc in range(NKC)
                    if classes[qt][kc][0] != "skip"
                ]
                nchunks = len(chunks)
                # av accumulators: [q=128, qc-sub, head, 65]; packed two
                # q-subchunks per PSUM bank
                av01 = psAV.tile([128, 2, 2, 65], f32, tag="av01",
                                 name=f"av01_{qt}_{p_}")
                av23 = psAV.tile([128, 2, 2, 65], f32, tag="av23",
                                 name=f"av23_{qt}_{p_}")
                avt = (av01, av01, av23, av23)
                # PSUM zero regions are whole 2 KiB banks: each av tile
                # (= one bank, two q-subchunks x two heads) gets exactly ONE
                # start=True (its first write zeroes the bank) and ONE
                # stop=True (its last write closes the group).
                bank_first = [None, None]   # first contributing chunk idx
                bank_last = [0, 0]          # last contributing chunk idx
                for idx, (kc, (cls, off)) in enumerate(chunks):
                    j0 = off // 128 if cls == "diag" else 0
                    for b in range(2):
                        if j0 <= 2 * b + 1:
                            if bank_first[b] is None:
                                bank_first[b] = idx
                            bank_last[b] = idx

                # per-bank normalize, emitted the moment the bank's
                # accumulation closes: the DVE chain overlaps the remaining
                # AV matmuls of the other bank
                ctx = spool.tile([128, 4, 2, HD], bf16, tag="ctxq",
                                 name=f"ctx{qt}_{p_}")

                def norm_bank(b):
                    av = avt[2 * b]
                    den = spool.tile([128, 2, 2], f32, tag="denom")
                    nc.vector.tensor_copy(den[:], av[:, :, :, 64])
                    rec = spool.tile([128, 2, 2], f32, tag="recip")
                    nc.vector.reciprocal_approx_fast(
                        out=rec.rearrange("p a b -> p (a b)"),
                        in_=den.rearrange("p a b -> p (a b)"),
                    )
                    nc.vector.tensor_mul(
                        ctx[:, 2 * b:2 * b + 2, :, :], av[:, :, :, 0:HD],
                        rec[:, :, :, None].to_broadcast((128, 2, 2, HD)),
                    )

                def emit_av(idx, off, kc, att):
                    j0 = off // 128
                    writes = [(qc, i) for qc in range(j0, 4)
                              for i in range(2)]
                    bfirst = {b: next(((q, i) for q, i in writes
                                       if q // 2 == b), None)
                              for b in range(2)}
                    blast = {b: next(((q, i) for q, i in reversed(writes)
                                      if q // 2 == b), None)
                             for b in range(2)}
                    for qc, i in writes:
                        b = qc // 2
                        stop = (idx == bank_last[b]
                                and (qc, i) == blast[b])
                        nc.tensor.matmul(
                            avt[qc][:, qc % 2, i, :],
                            att[:, i, qc * 128:(qc + 1) * 128],
                            v_sb[:, kc, 2 * p_ + i, :],
                            start=(idx == bank_first[b]
                                   and (qc, i) == bfirst[b]),
                            stop=stop,
                        )


                pend = deque()
                for idx, (kc, (cls, off)) in enumerate(chunks):
                    pairps = psQK.tile([128, 1024], f32, tag="qk",
                                       name=f"qk{qt}_{p_}_{kc}")
                    pq = pairps.rearrange("p (h q) -> p h q", h=2)
                    dg = cls == "diag"
                    for i in range(2):
                        lo = 64 * i
                        nc.tensor.matmul(
                            pq[:, i, off:QT],
                            kT_sb[lo:lo + 64, p_, kc * KC:(kc + 1) * KC],
                            qT_sb[lo:lo + 64, p_, q0 + off:q0 + QT],
                            start=True, stop=True,
                        )
                    if cls == "gen":
                        mt = spool.tile([128, QT], f32, tag="genmask")
                        nc.sync.dma_start(
                            mt[:], maskT_d[kc * KC:(kc + 1) * KC, q0:q0 + QT]
                        )
                        nc.vector.tensor_add(
                            pq[:, :, :], pq[:, :, :],
                            mt[:, None, :].to_broadcast((128, 2, QT)),
                        )
                    at = apool.tile([128, 1024], bf16, tag="attnT")
                    att = at.rearrange("p (h q) -> p h q", h=2)
                    nc.scalar.activation(
                        att[:, :, off:], pq[:, :, off:], FT.Exp,
                        bias=pad_sb[:, kc:kc + 1], scale=SCALE / 1024.0,
                    )
                    if dg:
                        # zero the sub-diagonal of the boundary block with a
                        # plain DVE multiply by a 0/1 tril mask (keep
                        # att[p, i, off+j] iff j >= p)
                        nc.vector.tensor_mul(
                            att[:, :, off:off + 128],
                            att[:, :, off:off + 128],
                            tril_sb[:, None, :].to_broadcast((128, 2, 128)),
                        )
                    # weave one group of independent PE work between the exp
                    # and its AV consumers so the PE stream has ready work
                    # while the activation engine computes the exp
                    if filler:
                        filler.popleft()()
                    # software-pipeline by two chunks: the AV matmuls for
                    # chunk i are emitted after chunk i+2's QK/exp, so the
                    # ~1.2us exp latency is fully covered by in-stream PE work
                    if len(pend) == 8:
                        emit_av(*pend.popleft())
                    pend.append((idx, off, kc, att))
                while pend:
                    if filler:
                        filler.popleft()()
                    emit_av(*pend.popleft())
                norm_bank(0)
                norm_bank(1)

                # ---- transpose ctx [q, ch] -> ctxT [ch, q] on the PE.
                # Returned as a deferred closure: the PE queue is in-order,
                # so emitting the transposes here would stall it on the DVE
                # normalize chain; the caller interleaves them into the next
                # attention block instead.
                def finish():
                    trp = psA.tile([128, 512], bf16, tag="proj",
                                   name=f"tr{qt}_{p_}")
                    c2 = ctx.rearrange("p q h d -> p q (h d)")
                    for qc in range(4):
                        nc.tensor.matmul(
                            trp[:, qc * 128:(qc + 1) * 128],
                            c2[:, qc, :], ident_sb[:],
                            is_transpose=True,
                            start=(qc == 0), stop=(qc == 3),
                        )
                    nc.vector.tensor_copy(
                        ctxT_sb[:, p_, q0:q0 + QT], trp[:]
                    )

                return finish

            def wo_g(qt, m, tail=False):
                q0 = qt * QT
                # drain-phase groups rotate over four PSUM rings (proj,
                # qk, and both idle av banks) so the Wo accumulations never
                # wait on a copy two slots back
                if tail is False:
                    pool_, tag = psA, "proj"
                else:
                    pool_, tag = ((psA, "proj"), (psQK, "qk"),
                                  (psAV, "av01"), (psAV, "av23"))[m % 4]
                po = pool_.tile([128, 512], f32, tag=tag, name=f"po{qt}{m}")
                for kc2 in range(2):
                    nc.tensor.matmul(
                        po[:],
                        wo_sb[:, kc2, m * 128:(m + 1) * 128],
                        ctxT_sb[:, kc2, q0:q0 + QT],
                        start=(kc2 == 0), stop=(kc2 == 1),
                    )
                if tail is not False:
                    # alternate ACT/DVE so no single copy queue serializes
                    # the drain; merge stores in m-pairs to halve descriptor
                    # time (GPSIMD cannot read PSUM on hardware)
                    eng = (nc.scalar.copy, nc.vector.tensor_copy)[m % 2]
                    eng(tail[:, m % 2, :], po[:])
                    if m % 2:
                        nc.sync.dma_start(
                            out_d[(m - 1) * 128:(m + 1) * 128, q0:q0 + QT]
                            .rearrange("(j p) q -> p j q", j=2),
                            tail[:],
                        )
                    return
                ot = spool.tile([128, 512], bf16, tag="wout")
                nc.vector.tensor_copy(ot[:], po[:])
                nc.sync.dma_start(
                    out_d[m * 128:(m + 1) * 128, q0:q0 + QT], ot[:]
                )

            # ---------- schedule ----------
            def pk(m, s4):
                return lambda: proj_qk_g(wk_sb, bk_sb, xk_sb, kT_sb, m, s4)

            def pq(m, s4):
                return lambda: proj_qk_g(wq_sb, bq_sb, xq_sb, qT_sb, m, s4)

            def pv_(st):
                return lambda: proj_v_g(st)

            # pair 0 attention (ascending qtiles) pipelines against the
            # remaining projection groups, fed one per attention chunk.
            # Emission order IS program order for Tile: every tile write
            # must be emitted before its first (program-order) reader, so
            # the K/Q m0 projections a qtile depends on are emitted right
            # before it and each V s-tile strictly before the chunk whose
            # AV matmul consumes it.

            def wof(qt):
                return [(lambda q, m: lambda: wo_g(q, m))(qt, m)
                        for m in range(8)]

            # the pipelined schedule needs qtile qt to touch only kchunks
            # <= 4*qt+3 (true for causal masks); otherwise emit everything
            # up front in dependency-safe order
            def pipelined_precheck():
                max_kc = [
                    max((kc for kc in range(NKC)
                         if classes[qt][kc][0] != "skip"), default=-1)
                    for qt in range(NQT)
                ]
                return all(max_kc[qt] <= 4 * qt + 3 for qt in range(NQT))

            pipelined = pipelined_precheck()
            fill = deque()
            noop = lambda: None
            if not pipelined:
                fill += [pv_(0), pv_(1), pv_(2), pv_(3)]
                fill += [pk(1, 0), pv_(4), pq(1, 0), pv_(5),
                         pk(1, 1), pv_(6), pq(1, 1), pv_(7)]
                fill += [noop, noop, noop, noop,
                         pv_(8), pv_(9), pv_(10), pv_(11)]
            wo_fill = deque()


            if pipelined:
                # interleave pair-1 blocks between pair-0 blocks: the exp
                # stream saturates the activation engine from ~10us on while
                # projection/Wo groups keep the PE dense via the filler queue
                q = wo_fill
                proj_qk_g(wk_sb, bk_sb, xk_sb, kT_sb, 0, 0)
                proj_qk_g(wk_sb, bk_sb, xk_sb, kT_sb, 1, 0)
                proj_qk_g(wq_sb, bq_sb, xq_sb, qT_sb, 0, 0)
                proj_qk_g(wq_sb, bq_sb, xq_sb, qT_sb, 1, 0)
                q += [pv_(0), pv_(1), pv_(2), pv_(3)]
                f00 = attn(0, 0, q)
                # pair-1 of qtile 0 next: it needs NO new DMA (the m1
                # projections were emitted during the startup window), so it
                # covers the xk1/xq1 load that qtile 1 pair 0 waits on
                q += [f00]
                f01 = attn(0, 1, q)
                proj_qk_g(wk_sb, bk_sb, xk_sb, kT_sb, 0, 1)
                proj_qk_g(wq_sb, bq_sb, xq_sb, qT_sb, 0, 1)
                q += [pv_(4), f01, pv_(5), pk(1, 1), pv_(6),
                      pq(1, 1), pv_(7), pk(0, 2), pq(0, 2)]
                f10 = attn(1, 0, q)
                w0 = wof(0)
                q += [pv_(8), f10, pv_(9), pv_(10), pv_(11),
                      pk(1, 2), pq(1, 2)] + w0[:5]
                f20 = attn(2, 0, q)
                q += w0[5:] + [f20, pk(0, 3), pq(0, 3)]
                f11 = attn(1, 1, q)
                w1 = wof(1)
                q += [f11] + w1
                f21 = attn(2, 1, q)
                w2 = wof(2)
                q += [pv_(12), f21, pv_(13), pv_(14), pv_(15),
                      pq(1, 3), w2[0], w2[1]]
                f30 = attn(3, 0, q)
                q += [w2[2], pk(1, 3), w2[3], f30] + w2[4:]
                f31 = attn(3, 1, q)
            else:
                for m in range(2):
                    for s4 in range(NQT):
                        proj_qk_g(wk_sb, bk_sb, xk_sb, kT_sb, m, s4)
                        proj_qk_g(wq_sb, bq_sb, xq_sb, qT_sb, m, s4)
                while fill:
                    fill.popleft()()  # V projections s0-11 and m1 leftovers
                for st in range(12, NKC):
                    proj_v_g(st)
                for qt in range(NQT):
                    f = attn(qt, 0, wo_fill)
                    wo_fill.append(f)
                for qt in range(NQT):
                    f = attn(qt, 1, wo_fill)
                    wo_fill.append(f)
                    if qt < NQT - 1:
                        wo_fill += wof(qt)
                f31 = None
            while wo_fill:  # emit unconsumed fillers (incl. any finishes)
                wo_fill.popleft()()
            if f31 is not None:
                f31()
            for m in range(0, 8, 2):
                ot2 = spool.tile([128, 2, 512], bf16, tag="wout2",
                                 name=f"ot2_{m}")
                wo_g(NQT - 1, m, tail=ot2)
                wo_g(NQT - 1, m + 1, tail=ot2)

    nc.compile()
    return nc


def _get_nc(classes, has_gen, skip_bias):
    key = (classes, has_gen, skip_bias)
    if key not in _NC_CACHE:
        _NC_CACHE[key] = _build(classes, has_gen, skip_bias)
    return _NC_CACHE[key]


def _xshard(x):  # [S, D] f32 -> [4, 128, 2, 8, 512] fp8 hi/lo (x^T tiles)
    xt = np.ascontiguousarray(np.asarray(x, F32).T)          # [D, S]
    a = xt.reshape(8, 128, NQT, QT).transpose(2, 1, 0, 3)    # [4, 128, 8, 512]
    hi, lo = _hilo(np.ascontiguousarray(a))
    return np.ascontiguousarray(np.stack([hi, lo], axis=2))  # [4,128,2,8,512]


def _vshard(x):  # [S, D] f32 -> [16, 128, 2, 8, 128] fp8 hi/lo
    xt = np.ascontiguousarray(np.asarray(x, F32).T)          # [D, S]
    a = xt.reshape(8, 128, NKC, KC).transpose(2, 1, 0, 3)    # [16, 128, 8, 128]
    hi, lo = _hilo(np.ascontiguousarray(a))
    return np.ascontiguousarray(np.stack([hi, lo], axis=2))


def _wshard(W, g):  # Linear weight [D, D] -> lhsT tiles [128, 2, 8, 256] fp8
    Wt = np.asarray(W, F32).T[:, g * DCH:(g + 1) * DCH] * WSCALE  # [D, 256]
    a = Wt.reshape(8, 128, DCH).transpose(1, 0, 2)
    hi, lo = _hilo(np.ascontiguousarray(a))
    return np.ascontiguousarray(np.stack([hi, lo], axis=1))


def _woshard(W, g):  # Wo [D, D] -> [128, 2, D] bf16 (rows = this core's ch)
    Wt = np.asarray(W, F32).T[g * DCH:(g + 1) * DCH, :] / WSCALE  # [256, D]
    return np.ascontiguousarray(
        Wt.reshape(2, 128, D).transpose(1, 0, 2)
    ).astype(BF16)


def _prep_in_maps(inputs, has_gen):
    pm = np.asarray(inputs["padding_mask"], F32)
    tril_np = np.where(
        np.arange(128)[:, None] <= np.arange(128)[None, :], 1.0, 0.0
    ).astype(BF16)
    ident_np = np.eye(128, dtype=np.float32).astype(BF16)
    maskT = None
    if has_gen:
        # the kernel folds the 1/(8*1024) logit scale into exp *after* the
        # mask add, so pre-scale the mask by 8192 to compensate
        maskT = np.ascontiguousarray(
            np.asarray(inputs["attention_mask"], F32).T * 8192.0
        )

    xs = {n: [_xshard(np.asarray(inputs[n], F32)[b]) for b in range(B)]
          for n in ("q", "k")}
    xs["v"] = [_vshard(np.asarray(inputs["v"], F32)[b]) for b in range(B)]
    ws = {n: [_wshard(inputs[w], g) for g in range(GROUPS)]
          for n, w in (("wq", "Wq"), ("wk", "Wk"), ("wv", "Wv"))}
    wos = [_woshard(inputs["Wo"], g) for g in range(GROUPS)]
    bs = {n: (np.asarray(inputs[b], F32) * WSCALE)
          .reshape(GROUPS, 1, DCH).astype(BF16)
          for n, b in (("bq", "bq"), ("bk", "bk"), ("bv", "bv"))}
    pads = [
        np.ascontiguousarray(pm[b].reshape(NKC, 128).T).astype(F32)
        for b in range(B)
    ]

    in_maps = []
    for c in range(NCORES):
        b, g = divmod(c, GROUPS)
        m = {
            "xq": xs["q"][b], "xk": xs["k"][b], "xv": xs["v"][b],
            "wq": ws["wq"][g], "wk": ws["wk"][g], "wv": ws["wv"][g],
            "wo": wos[g],
            "bq": bs["bq"][g], "bk": bs["bk"][g], "bv": bs["bv"][g],
            "pad": pads[b],
            "tril": tril_np,
            "ident": ident_np,
        }
        if has_gen:
            m["maskT"] = maskT
        in_maps.append(m)
    return in_maps


def _run(inputs, trace=False, **kw):
    mask = np.asarray(inputs["attention_mask"], F32)
    classes, has_gen = _classify(mask)
    skip_bias = not any(
        np.asarray(inputs[b], F32).any() for b in ("bq", "bk", "bv")
    )
    nc = _get_nc(classes, has_gen, skip_bias)
    in_maps = _prep_in_maps(inputs, has_gen)
    try:
        res = run_bass_kernel_spmd(
            nc, in_maps, list(range(NCORES)), trace=trace, **kw
        )
    except (ImportError, ModuleNotFoundError):
        # NTFF profiling hook unavailable in this container
        res = run_bass_kernel_spmd(
            nc, in_maps, list(range(NCORES)), trace=False, **kw
        )
    outs = np.zeros((B, S, D), F32)
    for c in range(NCORES):
        b, _ = divmod(c, GROUPS)
        outs[b] += np.asarray(res.results[c]["out"]).astype(F32).T
    outs += np.asarray(inputs["bo"], F32)[None, None, :]
    return outs, res


def kernel(**inputs) -> np.ndarray:
    out, _ = _run(inputs, trace=False)
    return out


# revision 6
# speedup vs baseline: 1.0014x; 1.0014x over previous
"""Trainium2 Bass kernel for nn_MultiHeadAttention (B=2, S=2048, D=1024, H=16).

Sharding: 8 cores = data-parallel over batch (2) x tensor-parallel over heads
(4 groups of 4 heads).  Each core:
  - computes Q^T, K^T (transposed [channels, seq] layout) and V (natural
    [seq, channels] layout, augmented with a ones column per head) for its
    4 heads via fp8 DoubleRow matmuls: inputs and weights are split hi/lo
    e4m3 and three of the four cross terms are accumulated (hi*hi, lo*hi,
    hi*lo), which beats bf16 in both speed (0.5 cycles/row, 2 contraction
    chunks per instruction) and accuracy (~0.1% proj error),
  - runs causal flash attention with *transposed* logits [k, q]; the AV
    product is computed in [q, d] orientation (att block as the stationary
    lhsT) so every AV matmul uses all 128 output partitions; the softmax
    denominator falls out of the ones column of V at free offset 64; AV
    emission is software-pipelined 8 chunks behind the QK/exp stream,
  - normalizes per-query on DVE (free-dim broadcast of the reciprocal),
    transposes ctx back to [ch, q] on the PE (identity-transpose matmuls,
    deferred into the next attention block as PE filler work),
  - multiplies by a row-sharded slice of Wo, producing a partial [D, S]
    output stored in bf16.
Host side: shards/pre-transposes inputs (weights pre-scaled x32 so fp8 lo
residuals stay in the normal range; Wo divided by 32 to compensate; the
1/(8*1024) logit scale is folded into the exp activation), sums the 4
partial Wo products per batch entry in f32 and adds the output bias.

HW-validated pitfalls baked into this design: GPSIMD cannot access PSUM
(all PSUM reads go through DVE/ACT); the TENSOR_MASK custom DVE op
mis-executes on HW (the causal boundary is a plain DVE multiply by a 0/1
tril mask post-exp instead); the custom-DVE reciprocal needs SBUF
partition-0 sources.
"""

import sys

for _p in ("/opt/trn_rl_repo", "/root/.axon_site/_ro/trn_rl_repo"):
    if _p not in sys.path:
        sys.path.insert(0, _p)

import numpy as np
import ml_dtypes

import concourse.bass as bass  # noqa: F401  (registers engines)
import concourse.mybir as mybir
import concourse.tile as tile
from concourse import bacc
from concourse.bass_utils import run_bass_kernel_spmd

BF16 = ml_dtypes.bfloat16
FP8 = ml_dtypes.float8_e4m3
F32 = np.float32
WSCALE = 32.0   # weight pre-scale keeping fp8 lo-residuals in normal range


def _hilo(a):
    """Split f32 array into (hi, lo) fp8 e4m3 with lo = a - f32(hi)."""
    hi = a.astype(FP8)
    lo = (a - hi.astype(F32)).astype(FP8)
    return hi, lo

B, S, D, H, HD = 2, 2048, 1024, 16, 64
NCORES = 8
GROUPS = NCORES // B        # 4 head groups
HPG = H // GROUPS           # 4 heads per core
DCH = HPG * HD              # 256 channels per core
QT = 512                    # query tile width (free dim)
KC = 128                    # key chunk (partition dim)
NQT, NKC = S // QT, S // KC  # 4, 16
SCALE = 1.0 / 8.0           # 1/sqrt(HD)
NEG_BIG = -1.0e9

_NC_CACHE: dict = {}


def _classify(mask: np.ndarray):
    """Classify each (qtile, kchunk) block of the additive attention mask.

    Returns tuple-of-tuples of (kind, off) with kind in
    {skip, full, diag, gen}; off is the first visible column for diag blocks.
    """
    classes = []
    for qt in range(NQT):
        q0 = qt * QT
        row = []
        for kc in range(NKC):
            k0 = kc * KC
            blk = mask[q0:q0 + QT, k0:k0 + KC]
            if np.all(blk <= -1e8):
                row.append(("skip", 0))
            elif not blk.any():
                row.append(("full", 0))
            else:
                off = k0 - q0
                if 0 <= off < QT:
                    qi = np.arange(q0, q0 + QT)[:, None]
                    ki = np.arange(k0, k0 + KC)[None, :]
                    vis = qi >= ki
                    if (not blk[vis].any()) and np.all(blk[~vis] <= -1e8):
                        row.append(("diag", off))
                        continue
                row.append(("gen", 0))
        classes.append(tuple(row))

    has_gen = any(c[0] == "gen" for r in classes for c in r)
    if has_gen:
        # keep things simple/correct for odd masks: every non-skip block
        # takes the general (full-width + mask add) path
        classes = [
            tuple(("gen", 0) if c[0] in ("diag", "full") else c for c in r)
            for r in classes
        ]
    # first visible chunk of each qtile must cover the full tile width so the
    # accumulating matmul's start=True pass initializes every column
    for r in classes:
        first = next((c for c in r if c[0] != "skip"), None)
        assert first is None or first[1] == 0, "unsupported mask pattern"
    return tuple(tuple(r) for r in classes), has_gen


def _build(classes, has_gen: bool, skip_bias: bool = True):
    f32, bf16 = mybir.dt.float32, mybir.dt.bfloat16
    FT = mybir.ActivationFunctionType

    nc = bacc.Bacc("TRN2", target_bir_lowering=False, debug=False)

    fp8 = mybir.dt.float8e4
    # x inputs arrive pre-transposed, split hi/lo fp8 (error-compensated),
    # chunked into seq tiles; weights likewise (pre-scaled x32 on the host
    # so the lo residuals stay in fp8's normal range)
    xq_d = nc.dram_tensor("xq", [NQT, 128, 2, 8, QT], fp8,
                          kind="ExternalInput")
    xk_d = nc.dram_tensor("xk", [NQT, 128, 2, 8, QT], fp8,
                          kind="ExternalInput")
    xv_d = nc.dram_tensor("xv", [NKC, 128, 2, 8, KC], fp8,
                          kind="ExternalInput")
    wq_d = nc.dram_tensor("wq", [128, 2, 8, DCH], fp8, kind="ExternalInput")
    wk_d = nc.dram_tensor("wk", [128, 2, 8, DCH], fp8, kind="ExternalInput")
    wv_d = nc.dram_tensor("wv", [128, 2, 8, DCH], fp8, kind="ExternalInput")
    wo_d = nc.dram_tensor("wo", [128, 2, D], bf16, kind="ExternalInput")
    bq_d = nc.dram_tensor("bq", [1, DCH], bf16, kind="ExternalInput")
    bk_d = nc.dram_tensor("bk", [1, DCH], bf16, kind="ExternalInput")
    bv_d = nc.dram_tensor("bv", [1, DCH], bf16, kind="ExternalInput")
    pad_d = nc.dram_tensor("pad", [128, NKC], f32, kind="ExternalInput")
    tril_d = nc.dram_tensor("tril", [128, 128], bf16, kind="ExternalInput")
    ident_d = nc.dram_tensor("ident", [128, 128], bf16, kind="ExternalInput")
    maskT_d = None
    if has_gen:
        maskT_d = nc.dram_tensor("maskT", [S, S], f32, kind="ExternalInput")
    out_d = nc.dram_tensor("out", [D, S], bf16, kind="ExternalOutput")

    with tile.TileContext(nc) as tc:
        with (
            tc.tile_pool(name="cpool", bufs=1) as cpool,
            tc.tile_pool(name="spool", bufs=5) as spool,
            tc.tile_pool(name="apool", bufs=12) as apool,
            tc.tile_pool(name="psA", bufs=2, space="PSUM") as psA,
            tc.tile_pool(name="psQK", bufs=2, space="PSUM") as psQK,
            tc.tile_pool(name="psAV", bufs=1, space="PSUM") as psAV,
        ):
            # ---------- constants ----------
            # (DMAs issued after the pipeline-critical weight/x loads: every
            # DMA costs >=500ns of descriptor time on the shared engine)
            ones = cpool.tile([1, 512], bf16)
            nc.gpsimd.memset(ones[:], 1.0)
            tril_sb = cpool.tile([128, 128], bf16)
            ident_sb = cpool.tile([128, 128], bf16)
            pad_sb = cpool.tile([128, NKC], f32)
            bq_sb = cpool.tile([1, DCH], bf16)
            bk_sb = cpool.tile([1, DCH], bf16)
            bv_sb = cpool.tile([1, DCH], bf16)

            # ---------- weights / activations (x in 4 seq chunks) ----------
            def make_x(name):
                return [
                    cpool.tile([128, 2, 8, QT], fp8, name=f"{name}{s4}")
                    for s4 in range(NQT)
                ]

            xk_sb, xq_sb = make_x("xk"), make_x("xq")
            xv_sb = [
                cpool.tile([128, 2, 8, KC], fp8, name=f"xv{st}")
                for st in range(NKC)
            ]
            # interleave weight + input DMAs so each pipeline stage can
            # start as soon as its seq-chunk lands
            # split the first seq-chunk loads so the leading projection's
            # matmuls can start as soon as the first contraction half lands
            wk_sb = cpool.tile([128, 2, 8, DCH], fp8)
            nc.sync.dma_start(wk_sb[:], wk_d[:])
            nc.sync.dma_start(xk_sb[0][:, 0, :, :], xk_d[0, :, 0, :, :])
            nc.sync.dma_start(xk_sb[0][:, 1, :, :], xk_d[0, :, 1, :, :])
            wq_sb = cpool.tile([128, 2, 8, DCH], fp8)
            nc.sync.dma_start(wq_sb[:], wq_d[:])
            nc.sync.dma_start(xq_sb[0][:, 0, :, :], xq_d[0, :, 0, :, :])
            nc.sync.dma_start(xq_sb[0][:, 1, :, :], xq_d[0, :, 1, :, :])
            wv_sb = cpool.tile([128, 2, 8, DCH], fp8)
            nc.sync.dma_start(wv_sb[:], wv_d[:])
            nc.gpsimd.dma_start(pad_sb[:], pad_d[:])
            nc.gpsimd.dma_start(tril_sb[:], tril_d[:])
            for st in range(4):
                nc.sync.dma_start(xv_sb[st][:], xv_d[st])
            nc.sync.dma_start(xk_sb[1][:], xk_d[1])
            nc.sync.dma_start(xq_sb[1][:], xq_d[1])
            nc.gpsimd.dma_start(ident_sb[:], ident_d[:])
            if not skip_bias:
                nc.gpsimd.dma_start(bq_sb[:], bq_d[:])
                nc.gpsimd.dma_start(bk_sb[:], bk_d[:])
                nc.gpsimd.dma_start(bv_sb[:], bv_d[:])
            nc.sync.dma_start(xv_sb[4][:], xv_d[4])
            nc.sync.dma_start(xv_sb[5][:], xv_d[5])
            nc.sync.dma_start(xv_sb[6][:], xv_d[6])
            nc.sync.dma_start(xv_sb[7][:], xv_d[7])
            for s4 in range(2, NQT):
                nc.sync.dma_start(xk_sb[s4][:], xk_d[s4])
                nc.sync.dma_start(xq_sb[s4][:], xq_d[s4])
                for st in range(4 * s4, 4 * s4 + 4):
                    nc.sync.dma_start(xv_sb[st][:], xv_d[st])
            wo_sb = cpool.tile([128, 2, D], bf16)
            nc.sync.dma_start(wo_sb[:], wo_d[:])

            qT_sb = cpool.tile([128, 2, S], bf16)   # [pair-stacked ch, pair, s]
            kT_sb = cpool.tile([128, 2, S], bf16)
            v_sb = cpool.tile([128, NKC, HPG, HD + 1], bf16)  # ones col at 64
            nc.gpsimd.memset(v_sb[:], 1.0)  # preset ones columns
            ctxT_sb = cpool.tile([128, 2, S], bf16)

            # ---------- projections (one PSUM group per call) ----------
            DR = mybir.MatmulPerfMode.DoubleRow
            TERMS = ((0, 0), (1, 0), (0, 1))  # (x hi/lo, w hi/lo)

            def proj_qk_g(w_sb, b_sb, x_sb, dst, m, s4):
                ps = psA.tile([128, 512], f32, tag="proj",
                              name=f"pp{id(w_sb) % 97}_{m}{s4}")
                n = 0
                for xt_, wt_ in TERMS:
                    for c2 in range(4):
                        nc.tensor.matmul(
                            ps[:],
                            w_sb[:, wt_, 2 * c2:2 * c2 + 2,
                                 m * 128:(m + 1) * 128],
                            x_sb[s4][:, xt_, 2 * c2:2 * c2 + 2, :],
                            start=(n == 0),
                            stop=(n == 11 and skip_bias),
                            perf_mode=DR,
                        )
                        n += 1
                if not skip_bias:
                    nc.tensor.matmul(
                        ps[:],
                        b_sb[0:1, m * 128:(m + 1) * 128],
                        ones[0:1, 0:512],
                        start=False, stop=True,
                    )
                nc.vector.tensor_copy(
                    dst[:, m, s4 * 512:(s4 + 1) * 512], ps[:]
                )

            def proj_v_g(st):
                ps = psA.tile([128, 512], f32, tag="proj", name=f"pv{st}")
                pv = ps[:, 0:DCH]
                xt = xv_sb[st]
                n = 0
                for xt_, wt_ in TERMS:
                    for c2 in range(4):
                        nc.tensor.matmul(
                            pv,
                            xt[:, xt_, 2 * c2:2 * c2 + 2, :],
                            wv_sb[:, wt_, 2 * c2:2 * c2 + 2, :],
                            start=(n == 0),
                            stop=(n == 11 and skip_bias),
                            perf_mode=DR,
                        )
                        n += 1
                if not skip_bias:
                    nc.tensor.matmul(
                        pv, ones[0:1, 0:128], bv_sb[0:1, :],
                        start=False, stop=True,
                    )
                # NB: ACT mis-executes this strided 3D copy; keep it on DVE
                nc.vector.tensor_copy(
                    v_sb[:, st, :, 0:HD],
                    ps[:, 0:DCH].rearrange("p (h d) -> p h d", h=HPG),
                )

            # ---------- attention for one (qtile, pair) ----------
            from collections import deque

            def attn(qt, p_, filler=None):
                q0 = qt * QT
                chunks = [
                    (kc, classes[qt][kc])
                    for kc in range(NKC)
                    if classes[qt][kc][0] != "skip"
                ]
                nchunks = len(chunks)
                # av accumulators: [q=128, qc-sub, head, 65]; packed two
                # q-subchunks per PSUM bank
                av01 = psAV.tile([128, 2, 2, 65], f32, tag="av01",
                                 name=f"av01_{qt}_{p_}")
                av23 = psAV.tile([128, 2, 2, 65], f32, tag="av23",
                                 name=f"av23_{qt}_{p_}")
                avt = (av01, av01, av23, av23)
                # PSUM zero regions are whole 2 KiB banks: each av tile
                # (= one bank, two q-subchunks x two heads) gets exactly ONE
                # start=True (its first write zeroes the bank) and ONE
                # stop=True (its last write closes the group).
                bank_first = [None, None]   # first contributing chunk idx
                bank_last = [0, 0]          # last contributing chunk idx
                for idx, (kc, (cls, off)) in enumerate(chunks):
                    j0 = off // 128 if cls == "diag" else 0
                    for b in range(2):
                        if j0 <= 2 * b + 1:
                            if bank_first[b] is None:
                                bank_first[b] = idx
                            bank_last[b] = idx

                # per-bank normalize, emitted the moment the bank's
                # accumulation closes: the DVE chain overlaps the remaining
                # AV matmuls of the other bank
                ctx = spool.tile([128, 4, 2, HD], bf16, tag="ctxq",
                                 name=f"ctx{qt}_{p_}")

                def norm_bank(b):
                    av = avt[2 * b]
                    den = spool.tile([128, 2, 2], f32, tag="denom")
                    nc.vector.tensor_copy(den[:], av[:, :, :, 64])
                    rec = spool.tile([128, 2, 2], f32, tag="recip")
                    nc.vector.reciprocal_approx_fast(
                        out=rec.rearrange("p a b -> p (a b)"),
                        in_=den.rearrange("p a b -> p (a b)"),
                    )
                    nc.vector.tensor_mul(
                        ctx[:, 2 * b:2 * b + 2, :, :], av[:, :, :, 0:HD],
                        rec[:, :, :, None].to_broadcast((128, 2, 2, HD)),
                    )

                def emit_av(idx, off, kc, att):
                    j0 = off // 128
                    writes = [(qc, i) for qc in range(j0, 4)
                              for i in range(2)]
                    bfirst = {b: next(((q, i) for q, i in writes
                                       if q // 2 == b), None)
                              for b in range(2)}
                    blast = {b: next(((q, i) for q, i in reversed(writes)
                                      if q // 2 == b), None)
                             for b in range(2)}
                    for qc, i in writes:
                        b = qc // 2
                        stop = (idx == bank_last[b]
                                and (qc, i) == blast[b])
                        nc.tensor.matmul(
                            avt[qc][:, qc % 2, i, :],
                            att[:, i, qc * 128:(qc + 1) * 128],
                            v_sb[:, kc, 2 * p_ + i, :],
                            start=(idx == bank_first[b]
                                   and (qc, i) == bfirst[b]),
                            stop=stop,
                        )


                pend = deque()
                for idx, (kc, (cls, off)) in enumerate(chunks):
                    pairps = psQK.tile([128, 1024], f32, tag="qk",
                                       name=f"qk{qt}_{p_}_{kc}")
                    pq = pairps.rearrange("p (h q) -> p h q", h=2)
                    dg = cls == "diag"
                    for i in range(2):
                        lo = 64 * i
                        nc.tensor.matmul(
                            pq[:, i, off:QT],
                            kT_sb[lo:lo + 64, p_, kc * KC:(kc + 1) * KC],
                            qT_sb[lo:lo + 64, p_, q0 + off:q0 + QT],
                            start=True, stop=True,
                        )
                    if cls == "gen":
                        mt = spool.tile([128, QT], f32, tag="genmask")
                        nc.sync.dma_start(
                            mt[:], maskT_d[kc * KC:(kc + 1) * KC, q0:q0 + QT]
                        )
                        nc.vector.tensor_add(
                            pq[:, :, :], pq[:, :, :],
                            mt[:, None, :].to_broadcast((128, 2, QT)),
                        )
                    at = apool.tile([128, 1024], bf16, tag="attnT")
                    att = at.rearrange("p (h q) -> p h q", h=2)
                    nc.scalar.activation(
                        att[:, :, off:], pq[:, :, off:], FT.Exp,
                        bias=pad_sb[:, kc:kc + 1], scale=SCALE / 1024.0,
                    )
                    if dg:
                        # zero the sub-diagonal of the boundary block with a
                        # plain DVE multiply by a 0/1 tril mask (keep
                        # att[p, i, off+j] iff j >= p)
                        nc.vector.tensor_mul(
                            att[:, :, off:off + 128],
                            att[:, :, off:off + 128],
                            tril_sb[:, None, :].to_broadcast((128, 2, 128)),
                        )
                    # weave one group of independent PE work between the exp
                    # and its AV consumers so the PE stream has ready work
                    # while the activation engine computes the exp
                    if filler:
                        filler.popleft()()
                    # software-pipeline by two chunks: the AV matmuls for
                    # chunk i are emitted after chunk i+2's QK/exp, so the
                    # ~1.2us exp latency is fully covered by in-stream PE work
                    if len(pend) == 8:
                        emit_av(*pend.popleft())
                    pend.append((idx, off, kc, att))
                while pend:
                    if filler:
                        filler.popleft()()
                    emit_av(*pend.popleft())
                norm_bank(0)
                norm_bank(1)

                # ---- transpose ctx [q, ch] -> ctxT [ch, q] on the PE.
                # Returned as a deferred closure: the PE queue is in-order,
                # so emitting the transposes here would stall it on the DVE
                # normalize chain; the caller interleaves them into the next
                # attention block instead.
                def finish():
                    trp = psA.tile([128, 512], bf16, tag="proj",
                                   name=f"tr{qt}_{p_}")
                    c2 = ctx.rearrange("p q h d -> p q (h d)")
                    for qc in range(4):
                        nc.tensor.matmul(
                            trp[:, qc * 128:(qc + 1) * 128],
                            c2[:, qc, :], ident_sb[:],
                            is_transpose=True,
                            start=(qc == 0), stop=(qc == 3),
                        )
                    nc.vector.tensor_copy(
                        ctxT_sb[:, p_, q0:q0 + QT], trp[:]
                    )

                return finish

            def wo_g(qt, m, tail=False):
                q0 = qt * QT
                # drain-phase groups rotate over four PSUM rings (proj,
                # qk, and both idle av banks) so the Wo accumulations never
                # wait on a copy two slots back
                if tail is False:
                    pool_, tag = psA, "proj"
                else:
                    pool_, tag = ((psA, "proj"), (psQK, "qk"),
                                  (psAV, "av01"), (psAV, "av23"))[m % 4]
                po = pool_.tile([128, 512], f32, tag=tag, name=f"po{qt}{m}")
                for kc2 in range(2):
                    nc.tensor.matmul(
                        po[:],
                        wo_sb[:, kc2, m * 128:(m + 1) * 128],
                        ctxT_sb[:, kc2, q0:q0 + QT],
                        start=(kc2 == 0), stop=(kc2 == 1),
                    )
                if tail is not False:
                    # alternate ACT/DVE so no single copy queue serializes
                    # the drain; merge stores in m-pairs to halve descriptor
                    # time (GPSIMD cannot read PSUM on hardware)
                    eng = (nc.scalar.copy, nc.vector.tensor_copy)[m % 2]
                    eng(tail[:, m % 2, :], po[:])
                    if m % 2:
                        nc.sync.dma_start(
                            out_d[(m - 1) * 128:(m + 1) * 128, q0:q0 + QT]
                            .rearrange("(j p) q -> p j q", j=2),
                            tail[:],
                        )
                    return
                ot = spool.tile([128, 512], bf16, tag="wout")
                nc.vector.tensor_copy(ot[:], po[:])
                nc.sync.dma_start(
                    out_d[m * 128:(m + 1) * 128, q0:q0 + QT], ot[:]
                )

            # ---------- schedule ----------
            def pk(m, s4):
                return lambda: proj_qk_g(wk_sb, bk_sb, xk_sb, kT_sb, m, s4)

            def pq(m, s4):
                return lambda: proj_qk_g(wq_sb, bq_sb, xq_sb, qT_sb, m, s4)

            def pv_(st):
                return lambda: proj_v_g(st)

            # pair 0 attention (ascending qtiles) pipelines against the
            # remaining projection groups, fed one per attention chunk.
            # Emission order IS program order for Tile: every tile write
            # must be emitted before its first (program-order) reader, so
            # the K/Q m0 projections a qtile depends on are emitted right
            # before it and each V s-tile strictly before the chunk whose
            # AV matmul consumes it.

            def wof(qt):
                return [(lambda q, m: lambda: wo_g(q, m))(qt, m)
                        for m in range(8)]

            # the pipelined schedule needs qtile qt to touch only kchunks
            # <= 4*qt+3 (true for causal masks); otherwise emit everything
            # up front in dependency-safe order
            def pipelined_precheck():
                max_kc = [
                    max((kc for kc in range(NKC)
                         if classes[qt][kc][0] != "skip"), default=-1)
                    for qt in range(NQT)
                ]
                return all(max_kc[qt] <= 4 * qt + 3 for qt in range(NQT))

            pipelined = pipelined_precheck()
            fill = deque()
            noop = lambda: None
            if not pipelined:
                fill += [pv_(0), pv_(1), pv_(2), pv_(3)]
                fill += [pk(1, 0), pv_(4), pq(1, 0), pv_(5),
                         pk(1, 1), pv_(6), pq(1, 1), pv_(7)]
                fill += [noop, noop, noop, noop,
                         pv_(8), pv_(9), pv_(10), pv_(11)]
            wo_fill = deque()


            if pipelined:
                # interleave pair-1 blocks between pair-0 blocks: the exp
                # stream saturates the activation engine from ~10us on while
                # projection/Wo groups keep the PE dense via the filler queue
                q = wo_fill
                proj_qk_g(wk_sb, bk_sb, xk_sb, kT_sb, 0, 0)
                proj_qk_g(wk_sb, bk_sb, xk_sb, kT_sb, 1, 0)
                proj_qk_g(wq_sb, bq_sb, xq_sb, qT_sb, 0, 0)
                proj_qk_g(wq_sb, bq_sb, xq_sb, qT_sb, 1, 0)
                q += [pv_(0), pv_(1), pv_(2), pv_(3)]
                f00 = attn(0, 0, q)
                # pair-1 of qtile 0 next: it needs NO new DMA (the m1
                # projections were emitted during the startup window), so it
                # covers the xk1/xq1 load that qtile 1 pair 0 waits on
                q += [f00]
                f01 = attn(0, 1, q)
                proj_qk_g(wk_sb, bk_sb, xk_sb, kT_sb, 0, 1)
                proj_qk_g(wq_sb, bq_sb, xq_sb, qT_sb, 0, 1)
                q += [pv_(4), f01, pv_(5), pk(1, 1), pv_(6),
                      pq(1, 1), pv_(7), pk(0, 2), pq(0, 2)]
                f10 = attn(1, 0, q)
                w0 = wof(0)
                q += [pv_(8), f10, pv_(9), pv_(10), pv_(11),
                      pk(1, 2), pq(1, 2)] + w0[:5]
                f20 = attn(2, 0, q)
                q += w0[5:] + [f20, pk(0, 3), pq(0, 3)]
                f11 = attn(1, 1, q)
                w1 = wof(1)
                q += [f11] + w1
                f21 = attn(2, 1, q)
                w2 = wof(2)
                q += [pv_(12), f21, pv_(13), pv_(14), pv_(15),
                      pq(1, 3), w2[0], w2[1]]
                f30 = attn(3, 0, q)
                q += [w2[2], pk(1, 3), w2[3], f30] + w2[4:]
                f31 = attn(3, 1, q)
            else:
                for m in range(2):
                    for s4 in range(NQT):
                        proj_qk_g(wk_sb, bk_sb, xk_sb, kT_sb, m, s4)
                        proj_qk_g(wq_sb, bq_sb, xq_sb, qT_sb, m, s4)
                while fill:
                    fill.popleft()()  # V projections s0-11 and m1 leftovers
                for st in range(12, NKC):
                    proj_v_g(st)
                for qt in range(NQT):
                    f = attn(qt, 0, wo_fill)
                    wo_fill.append(f)
                for qt in range(NQT):
                    f = attn(qt, 1, wo_fill)
                    wo_fill.append(f)
                    if qt < NQT - 1:
                        wo_fill += wof(qt)
                f31 = None
            while wo_fill:  # emit unconsumed fillers (incl. any finishes)
                wo_fill.popleft()()
            if f31 is not None:
                f31()
            for m in range(0, 8, 2):
                ot2 = spool.tile([128, 2, 512], bf16, tag="wout2",
                                 name=f"ot2_{m}")
                wo_g(NQT - 1, m, tail=ot2)
                wo_g(NQT - 1, m + 1, tail=ot2)

    nc.compile()
    return nc


def _get_nc(classes, has_gen, skip_bias):
    key = (classes, has_gen, skip_bias)
    if key not in _NC_CACHE:
        _NC_CACHE[key] = _build(classes, has_gen, skip_bias)
    return _NC_CACHE[key]


def _xshard(x):  # [S, D] f32 -> [4, 128, 2, 8, 512] fp8 hi/lo (x^T tiles)
    xt = np.ascontiguousarray(np.asarray(x, F32).T)          # [D, S]
    a = xt.reshape(8, 128, NQT, QT).transpose(2, 1, 0, 3)    # [4, 128, 8, 512]
    hi, lo = _hilo(np.ascontiguousarray(a))
    return np.ascontiguousarray(np.stack([hi, lo], axis=2))  # [4,128,2,8,512]


def _vshard(x):  # [S, D] f32 -> [16, 128, 2, 8, 128] fp8 hi/lo
    xt = np.ascontiguousarray(np.asarray(x, F32).T)          # [D, S]
    a = xt.reshape(8, 128, NKC, KC).transpose(2, 1, 0, 3)    # [16, 128, 8, 128]
    hi, lo = _hilo(np.ascontiguousarray(a))
    return np.ascontiguousarray(np.stack([hi, lo], axis=2))


def _wshard(W, g):  # Linear weight [D, D] -> lhsT tiles [128, 2, 8, 256] fp8
    Wt = np.asarray(W, F32).T[:, g * DCH:(g + 1) * DCH] * WSCALE  # [D, 256]
    a = Wt.reshape(8, 128, DCH).transpose(1, 0, 2)
    hi, lo = _hilo(np.ascontiguousarray(a))
    return np.ascontiguousarray(np.stack([hi, lo], axis=1))


def _woshard(W, g):  # Wo [D, D] -> [128, 2, D] bf16 (rows = this core's ch)
    Wt = np.asarray(W, F32).T[g * DCH:(g + 1) * DCH, :] / WSCALE  # [256, D]
    return np.ascontiguousarray(
        Wt.reshape(2, 128, D).transpose(1, 0, 2)
    ).astype(BF16)


def _prep_in_maps(inputs, has_gen):
    pm = np.asarray(inputs["padding_mask"], F32)
    tril_np = np.where(
        np.arange(128)[:, None] <= np.arange(128)[None, :], 1.0, 0.0
    ).astype(BF16)
    ident_np = np.eye(128, dtype=np.float32).astype(BF16)
    maskT = None
    if has_gen:
        # the kernel folds the 1/(8*1024) logit scale into exp *after* the
        # mask add, so pre-scale the mask by 8192 to compensate
        maskT = np.ascontiguousarray(
            np.asarray(inputs["attention_mask"], F32).T * 8192.0
        )

    xs = {n: [_xshard(np.asarray(inputs[n], F32)[b]) for b in range(B)]
          for n in ("q", "k")}
    xs["v"] = [_vshard(np.asarray(inputs["v"], F32)[b]) for b in range(B)]
    ws = {n: [_wshard(inputs[w], g) for g in range(GROUPS)]
          for n, w in (("wq", "Wq"), ("wk", "Wk"), ("wv", "Wv"))}
    wos = [_woshard(inputs["Wo"], g) for g in range(GROUPS)]
    bs = {n: (np.asarray(inputs[b], F32) * WSCALE)
          .reshape(GROUPS, 1, DCH).astype(BF16)
          for n, b in (("bq", "bq"), ("bk", "bk"), ("bv", "bv"))}
    pads = [
        np.ascontiguousarray(pm[b].reshape(NKC, 128).T).astype(F32)
        for b in range(B)
    ]

    in_maps = []
    for c in range(NCORES):
        b, g = divmod(c, GROUPS)
        m = {
            "xq": xs["q"][b], "xk": xs["k"][b], "xv": xs["v"][b],
            "wq": ws["wq"][g], "wk": ws["wk"][g], "wv": ws["wv"][g],
            "wo": wos[g],
            "bq": bs["bq"][g], "bk": bs["bk"][g], "bv": bs["bv"][g],
            "pad": pads[b],
            "tril": tril_np,
            "ident": ident_np,
        }
        if has_gen:
            m["maskT"] = maskT
        in_maps.append(m)
    return in_maps


def _run(inputs, trace=False, **kw):
    mask = np.asarray(inputs["attention_mask"], F32)
    classes, has_gen = _classify(mask)
    skip_bias = not any(
        np.asarray(inputs[b], F32).any() for b in ("bq", "bk", "bv")
    )
    nc = _get_nc(classes, has_gen, skip_bias)
    in_maps = _prep_in_maps(inputs, has_gen)
    try:
        res = run_bass_kernel_spmd(
            nc, in_maps, list(range(NCORES)), trace=trace, **kw
        )
    except (ImportError, ModuleNotFoundError):
        # NTFF profiling hook unavailable in this container
        res = run_bass_kernel_spmd(
            nc, in_maps, list(range(NCORES)), trace=False, **kw
        )
    outs = np.zeros((B, S, D), F32)
    for c in range(NCORES):
        b, _ = divmod(c, GROUPS)
        outs[b] += np.asarray(res.results[c]["out"]).astype(F32).T
    outs += np.asarray(inputs["bo"], F32)[None, None, :]
    return outs, res


def kernel(**inputs) -> np.ndarray:
    out, _ = _run(inputs, trace=False)
    return out
